# revision 6
# baseline (speedup 1.0000x reference)
"""Trainium2 Bass kernel: additive (Bahdanau-style) attention.

reference:
    proj_feat = features @ W1 + b1          # [B,T,D]
    proj_hid  = (hidden @ W2 + b2)[:,None]  # [B,1,D]
    ah        = tanh(proj_feat + proj_hid)  # [B,T,D]
    score     = ah @ Wv + bv                # [B,T,1]
    attn      = softmax(score, axis=1)      # [B,T,1]
    ctx       = sum(attn * features, 1)     # [B,F]
    return ctx, attn

Sharding: data-parallel on batch B=32 across 8 NeuronCores (4 per core),
weights replicated. No collectives.

Per-core dataflow (t-chunked, CHUNK=512):
  - load natural feature chunk fp32, cast to bf16 (gpsimd)
  - PE-transpose 128x128 subtiles -> featT (f on partitions)
  - mm1: psum[t,d-half] = ones⊗ph (K=1 bias matmul) + sum_kf featT.T @ W1
  - ACT tanh(psum) -> bf16
  - DVE scalar_tensor_tensor(tanh * Wv, accum) -> score col [128,1]
  - ACT exp(score cols); PE mm2: ctx_psum += exp_col.T @ nat_chunk
  - batch end: total = ones-matmul partition sum; attn = exp/total
    (PE-transposed to [16,128] for contiguous DMA out); ctx = ctx_psum/total
"""

import os
import sys
from contextlib import ExitStack

import numpy as np

sys.path.insert(0, "/opt/trn_rl_repo")

import concourse.bass as bass  # noqa: E402
import concourse.bacc as bacc  # noqa: E402
import concourse.tile as tile  # noqa: E402
import concourse.mybir as mybir  # noqa: E402
from concourse import masks  # noqa: E402
from concourse.bass_utils import run_bass_kernel_spmd  # noqa: E402

P = 128
F32 = mybir.dt.float32

# problem dims (full, host side)
B, T, F, H, DIM = 32, 2048, 1024, 1024, 1024
N_CORES = 8
B_L = B // N_CORES


def build_attention_kernel(
    nc,
    tc,
    ins: dict,
    outs: dict,
    B_L: int,
    T: int,
    F: int,
    H: int,
    D: int,
    CHUNK: int = 512,
    cdt=mybir.dt.bfloat16,
):
    """Emit the per-core kernel. ins/outs are dicts of DRAM APs."""
    ctx = ExitStack()
    KF = F // P          # k-tiles of the F contraction
    KH = H // P          # k-tiles of the H contraction
    ND2 = (D + 511) // 512   # 512-wide d-halves
    NF2 = (F + 511) // 512   # 512-wide f-halves
    ST = CHUNK // P      # t-subtiles per chunk
    NC_ = T // CHUNK     # chunks per batch
    TS = T // P          # score columns per batch
    ND512 = min(512, D)
    NF512 = min(512, F)

    feat_in = ins["features"]   # [B_L, T, F] f32
    hid_in_d = ins["hidden"]    # [B_L, H] f32
    w1_d = ins["W1"]            # [F, D]
    b1_d = ins["b1"]            # [D]
    w2_d = ins["W2"]            # [H, D]
    b2_d = ins["b2"]            # [D]
    wv_d = ins["Wv"]            # [D, 1]
    ctx_out = outs["ctx"]       # [B_L, F] f32
    attn_out = outs["attnw"]    # [B_L, T] f32

    # internal DRAM scratch for the (proj_hid + b1 + b2) rows
    ph_dram = nc.dram_tensor("ph_scratch", [B_L, D], F32).ap()

    # ---------------- pools ----------------
    const_pool = ctx.enter_context(tc.tile_pool(name="const", bufs=1))
    wpool = ctx.enter_context(tc.tile_pool(name="weights", bufs=1))
    wload = ctx.enter_context(tc.tile_pool(name="wload", bufs=2))
    natc_pool = ctx.enter_context(tc.tile_pool(name="natc", bufs=2))
    natbf_pool = ctx.enter_context(tc.tile_pool(name="natbf", bufs=3))
    featT_pool = ctx.enter_context(tc.tile_pool(name="featT", bufs=2))
    tanh_pool = ctx.enter_context(tc.tile_pool(name="tanh", bufs=3))
    ttrs_pool = ctx.enter_context(tc.tile_pool(name="ttrs", bufs=2))
    sm_pool = ctx.enter_context(tc.tile_pool(name="smallsb", bufs=2))
    batch_pool = ctx.enter_context(tc.tile_pool(name="batchsb", bufs=2))

    ps_tr = ctx.enter_context(tc.tile_pool(name="ps_tr", bufs=2, space="PSUM"))
    ps_mm = ctx.enter_context(tc.tile_pool(name="ps_mm", bufs=3, space="PSUM"))
    ps_ctx = ctx.enter_context(tc.tile_pool(name="ps_ctx", bufs=1, space="PSUM"))

    # ---------------- constants ----------------
    ident_cdt = const_pool.tile([P, P], cdt, name="ident_cdt")
    masks.make_identity(nc, ident_cdt[:])
    ident_f32 = const_pool.tile([P, P], F32, name="ident_f32")
    masks.make_identity(nc, ident_f32[:])
    ones_f32 = const_pool.tile([P, P], F32, name="ones_f32")
    nc.gpsimd.memset(ones_f32[:], 1.0)
    ones_row = const_pool.tile([1, P], cdt, name="ones_row")
    nc.gpsimd.memset(ones_row[:], 1.0)

    def load_weight_cdt(dram_ap, kt, name):
        """Load [P, D]-ish slices of a [K, D] weight, cast to cdt."""
        tiles = []
        for k in range(kt):
            src = dram_ap[k * P:(k + 1) * P, :]
            cols = src.shape[1]
            tmp = wload.tile([P, cols], F32)
            nc.sync.dma_start(tmp[:], src)
            wt = wpool.tile([P, cols], cdt, name=f"{name}_{k}")
            nc.vector.tensor_copy(wt[:], tmp[:])
            tiles.append(wt)
        return tiles

    w1c = load_weight_cdt(w1_d, KF, "w1")
    w2c = load_weight_cdt(w2_d, KH, "w2")

    # Wv broadcast across partitions: [P, D] in cdt
    wvb_f32 = const_pool.tile([P, D], F32, name="wvb_f32")
    nc.sync.dma_start(wvb_f32[:], wv_d.rearrange("d o -> o d").to_broadcast((P, D)))
    wvbc = const_pool.tile([P, D], cdt, name="wvbc")
    nc.vector.tensor_copy(wvbc[:], wvb_f32[:])

    # ---------------- stage 0: proj_hid = hidden @ W2 + b2 (+ b1) ----------------
    hid_sb = sm_pool.tile([B_L, H], F32, name="hid_sb")
    nc.sync.dma_start(hid_sb[:], hid_in_d[:, :])

    # transpose hidden -> hidT [P, B_L] per k-tile (cdt)
    hidT = []
    for kh in range(KH):
        pst = ps_tr.tile([P, B_L], F32, name="ps_tr")
        nc.tensor.matmul(
            pst[:], hid_sb[:, kh * P:(kh + 1) * P], ident_f32[:B_L, :B_L],
            is_transpose=True,
        )
        ht = sm_pool.tile([P, B_L], cdt, name=f"hidT_{kh}")
        nc.scalar.copy(ht[:], pst[:])
        hidT.append(ht)

    # proj_hid matmuls -> psum [B_L, 512] x ND2
    hidb = sm_pool.tile([B_L, D], F32, name="hidb")
    for nh in range(ND2):
        psph = ps_mm.tile([B_L, ND512], F32, name="ps_mm1")
        for kh in range(KH):
            nc.tensor.matmul(
                psph[:], hidT[kh][:], w2c[kh][:, nh * ND512:(nh + 1) * ND512],
                start=(kh == 0), stop=(kh == KH - 1),
            )
        nc.scalar.copy(hidb[:, nh * ND512:(nh + 1) * ND512], psph[:])

    # add b1 + b2 rows (broadcast-DMA'd to [B_L, D])
    for bias_d in (b1_d, b2_d):
        bb = sm_pool.tile([B_L, D], F32, name="biasb")
        nc.sync.dma_start(bb[:], bias_d[None, :].to_broadcast((B_L, D)))
        nc.vector.tensor_add(hidb[:], hidb[:], bb[:])

    nc.sync.dma_start(ph_dram[:, :], hidb[:])

    # ---------------- per-batch steady state ----------------
    for b in range(B_L):
        # bias row for this batch in cdt
        ph_f32 = sm_pool.tile([1, D], F32, name="ph_f32")
        nc.sync.dma_start(ph_f32[:], ph_dram[b:b + 1, :])
        phc = sm_pool.tile([1, D], cdt, name="phc")
        nc.vector.tensor_copy(phc[:], ph_f32[:])

        score_sb = batch_pool.tile([P, TS], F32, name="score")
        exps_sb = batch_pool.tile([P, TS], F32, name="exps")
        exps_cdt = batch_pool.tile([P, TS], cdt, name="exps_cdt")

        ctx_ps = [ps_ctx.tile([1, NF512], F32, name=f"ps_ctx{nf}") for nf in range(NF2)]

        natbf_tiles = []
        for c in range(NC_):
            # ---- load natural chunk, cast to cdt ----
            natc = natc_pool.tile([P, ST, F], F32, name="natc")
            nc.sync.dma_start(
                natc[:],
                feat_in[b, c * CHUNK:(c + 1) * CHUNK, :].rearrange(
                    "(st p) f -> p st f", p=P
                ),
            )
            natbf = natbf_pool.tile([P, ST, F], cdt, name="natbf")
            nc.gpsimd.tensor_copy(natbf[:], natc[:])
            natbf_tiles.append(natbf)

            # ---- transpose to featT [P(f), KF, CHUNK(t)] ----
            featT = featT_pool.tile([P, KF, CHUNK], cdt, name="featT")
            for kf in range(KF):
                pst = ps_tr.tile([P, CHUNK], cdt, name="ps_tr")
                for st in range(ST):
                    nc.tensor.matmul(
                        pst[:, st * P:(st + 1) * P],
                        natbf[:, st, kf * P:(kf + 1) * P],
                        ident_cdt[:],
                        is_transpose=True,
                    )
                nc.scalar.copy(featT[:, kf, :], pst[:])

            # ---- mm1 + tanh + score ----
            for st in range(ST):
                tanhc = tanh_pool.tile([P, D], cdt, name="tanhc")
                for nd in range(ND2):
                    pso = ps_mm.tile([P, ND512], F32, name="ps_mm1")
                    # K=1 bias matmul: ones[128] (x) ph_row -> broadcast add
                    nc.tensor.matmul(
                        pso[:], ones_row[:], phc[:, nd * ND512:(nd + 1) * ND512],
                        start=True, stop=False,
                    )
                    for kf in range(KF):
                        nc.tensor.matmul(
                            pso[:],
                            featT[:, kf, st * P:(st + 1) * P],
                            w1c[kf][:, nd * ND512:(nd + 1) * ND512],
                            start=False, stop=(kf == KF - 1),
                        )
                    nc.scalar.activation(
                        tanhc[:, nd * ND512:(nd + 1) * ND512], pso[:],
                        mybir.ActivationFunctionType.Tanh,
                    )
                # score col = sum_d tanh * Wv
                col = c * ST + st
                ttrs = ttrs_pool.tile([P, D], cdt, name="ttrs")
                nc.vector.scalar_tensor_tensor(
                    out=ttrs[:], in0=tanhc[:], scalar=0.0, in1=wvbc[:],
                    op0=mybir.AluOpType.bypass, op1=mybir.AluOpType.mult,
                    accum_out=score_sb[:, col:col + 1],
                )

            # ---- exp of this chunk's score cols ----
            nc.scalar.activation(
                exps_sb[:, c * ST:(c + 1) * ST], score_sb[:, c * ST:(c + 1) * ST],
                mybir.ActivationFunctionType.Exp,
            )
            nc.vector.tensor_copy(
                exps_cdt[:, c * ST:(c + 1) * ST], exps_sb[:, c * ST:(c + 1) * ST]
            )

            # ---- mm2 partial: ctx_psum += exp_col.T @ nat_chunk ----
            for st in range(ST):
                col = c * ST + st
                for nf in range(NF2):
                    nc.tensor.matmul(
                        ctx_ps[nf][:],
                        exps_cdt[:, col:col + 1],
                        natbf[:, st, nf * NF512:(nf + 1) * NF512],
                        start=(c == 0 and st == 0),
                        stop=(c == NC_ - 1 and st == ST - 1),
                    )

        # ---------------- batch epilogue: softmax normalize ----------------
        rowsum = sm_pool.tile([P, 1], F32, name="rowsum")
        nc.vector.reduce_sum(rowsum[:], exps_sb[:], axis=mybir.AxisListType.X)
        ps_tot = ps_tr.tile([P, 1], F32, name="ps_tr")
        nc.tensor.matmul(ps_tot[:], ones_f32[:], rowsum[:])
        rtotb = sm_pool.tile([P, 1], F32, name="rtotb")
        nc.vector.reciprocal(rtotb[:], ps_tot[:])

        # normalized attention weights, transposed for contiguous DMA out
        attnw = batch_pool.tile([P, TS], F32, name="attnw")
        nc.vector.tensor_scalar_mul(attnw[:], exps_sb[:], rtotb[:])
        ps_at = ps_tr.tile([TS, P], F32, name="ps_tr")
        nc.tensor.matmul(ps_at[:], attnw[:], ident_f32[:], is_transpose=True)
        at_sb = sm_pool.tile([TS, P], F32, name="at_sb")
        nc.scalar.copy(at_sb[:], ps_at[:])
        nc.sync.dma_start(
            attn_out[b].rearrange("(j p) -> j p", p=P), at_sb[:]
        )

        # ctx = ctx_psum / total
        ctx_sb = sm_pool.tile([1, F], F32, name="ctx_sb")
        for nf in range(NF2):
            nc.scalar.mul(
                ctx_sb[:, nf * NF512:(nf + 1) * NF512], ctx_ps[nf][:],
                rtotb[0:1, 0:1],
            )
        nc.sync.dma_start(ctx_out[b:b + 1, :], ctx_sb[:])

    ctx.close()


def build_nc(
    B_L=B_L, T=T, F=F, H=H, D=DIM, CHUNK=512, cdt=mybir.dt.bfloat16,
    tile_kwargs=None,
):
    """Build + schedule + compile the per-core Bass module."""
    nc = bacc.Bacc("TRN2", target_bir_lowering=False, debug=False,
                   num_devices=N_CORES)
    ins = {
        "hidden": nc.dram_tensor("hidden", [B_L, H], F32, kind="ExternalInput").ap(),
        "features": nc.dram_tensor("features", [B_L, T, F], F32, kind="ExternalInput").ap(),
        "W1": nc.dram_tensor("W1", [F, D], F32, kind="ExternalInput").ap(),
        "b1": nc.dram_tensor("b1", [D], F32, kind="ExternalInput").ap(),
        "W2": nc.dram_tensor("W2", [H, D], F32, kind="ExternalInput").ap(),
        "b2": nc.dram_tensor("b2", [D], F32, kind="ExternalInput").ap(),
        "Wv": nc.dram_tensor("Wv", [D, 1], F32, kind="ExternalInput").ap(),
        "bv": nc.dram_tensor("bv", [1], F32, kind="ExternalInput").ap(),
    }
    outs = {
        "ctx": nc.dram_tensor("ctx", [B_L, F], F32, kind="ExternalOutput").ap(),
        "attnw": nc.dram_tensor("attnw", [B_L, T], F32, kind="ExternalOutput").ap(),
    }
    with tile.TileContext(nc, **(tile_kwargs or {})) as tc:
        build_attention_kernel(nc, tc, ins, outs, B_L, T, F, H, D, CHUNK, cdt)
    nc.compile()
    return nc


_nc_cache = None

# test-harness knobs (the grading harness just calls kernel(); these stay default)
TRACE = False
LAST_RESULTS = None


def kernel(hidden, features, W1, b1, W2, b2, Wv, bv):
    global _nc_cache, LAST_RESULTS
    if _nc_cache is None:
        _nc_cache = build_nc()
    nc = _nc_cache

    hidden = np.ascontiguousarray(hidden, dtype=np.float32)
    features = np.ascontiguousarray(features, dtype=np.float32)
    in_common = {
        "W1": np.ascontiguousarray(W1, np.float32),
        "b1": np.ascontiguousarray(b1, np.float32),
        "W2": np.ascontiguousarray(W2, np.float32),
        "b2": np.ascontiguousarray(b2, np.float32),
        "Wv": np.ascontiguousarray(Wv, np.float32),
        "bv": np.ascontiguousarray(bv, np.float32),
    }
    in_maps = []
    for i in range(N_CORES):
        sl = slice(i * B_L, (i + 1) * B_L)
        in_maps.append({
            "hidden": hidden[sl], "features": features[sl], **in_common,
        })

    res = run_bass_kernel_spmd(nc, in_maps, core_ids=list(range(N_CORES)),
                               trace=TRACE)
    LAST_RESULTS = res
    ctx = np.concatenate([r["ctx"] for r in res.results], axis=0)
    attn = np.concatenate([r["attnw"] for r in res.results], axis=0)[..., None]
    return ctx.astype(np.float32), attn.astype(np.float32)


# revision 15
# speedup vs baseline: 1.2003x; 1.2003x over previous
"""Trainium2 Bass kernel: additive (Bahdanau-style) attention.

reference:
    proj_feat = features @ W1 + b1          # [B,T,D]
    proj_hid  = (hidden @ W2 + b2)[:,None]  # [B,1,D]
    ah        = tanh(proj_feat + proj_hid)  # [B,T,D]
    score     = ah @ Wv + bv                # [B,T,1]
    attn      = softmax(score, axis=1)      # [B,T,1]
    ctx       = sum(attn * features, 1)     # [B,F]
    return ctx, attn

Sharding: data-parallel on batch B=32 across 8 NeuronCores (4 per core),
weights replicated. No collectives.

Per-core dataflow (t-chunked, CHUNK=512):
  - load natural feature chunk fp32, cast to bf16 (gpsimd)
  - PE-transpose 128x128 subtiles -> featT (f on partitions)
  - mm1: psum[t,d-half] = ones⊗ph (K=1 bias matmul) + sum_kf featT.T @ W1
  - ACT tanh(psum) -> bf16
  - DVE scalar_tensor_tensor(tanh * Wv, accum) -> score col [128,1]
  - ACT exp(score cols); PE mm2: ctx_psum += exp_col.T @ nat_chunk
  - batch end: total = ones-matmul partition sum; attn = exp/total
    (PE-transposed to [16,128] for contiguous DMA out); ctx = ctx_psum/total
"""

import os
import sys
from contextlib import ExitStack

import numpy as np

sys.path.insert(0, "/opt/trn_rl_repo")

import concourse.bass as bass  # noqa: E402
import concourse.bacc as bacc  # noqa: E402
import concourse.tile as tile  # noqa: E402
import concourse.mybir as mybir  # noqa: E402
from concourse import masks  # noqa: E402
from concourse.bass_utils import run_bass_kernel_spmd  # noqa: E402

P = 128
F32 = mybir.dt.float32

# problem dims (full, host side)
B, T, F, H, DIM = 32, 2048, 1024, 1024, 1024
N_CORES = 8
B_L = B // N_CORES


def build_attention_kernel(
    nc,
    tc,
    ins: dict,
    outs: dict,
    B_L: int,
    T: int,
    F: int,
    H: int,
    D: int,
    CHUNK: int = 512,
    cdt=mybir.dt.bfloat16,
):
    """Emit the per-core kernel. ins/outs are dicts of DRAM APs."""
    ctx = ExitStack()
    KF = F // P          # k-tiles of the F contraction
    KH = H // P          # k-tiles of the H contraction
    ND2 = (D + 511) // 512   # 512-wide d-halves
    NF2 = (F + 511) // 512   # 512-wide f-halves
    ST = CHUNK // P      # t-subtiles per chunk
    NC_ = T // CHUNK     # chunks per batch
    TS = T // P          # score columns per batch
    ND512 = min(512, D)
    NF512 = min(512, F)

    feat_in = ins["features"]   # [B_L, T, F] f32
    hid_in_d = ins["hidden"]    # [B_L, H] f32
    w1_d = ins["W1"]            # [F, D]
    b1_d = ins["b1"]            # [D]
    w2_d = ins["W2"]            # [H, D]
    b2_d = ins["b2"]            # [D]
    wv_d = ins["Wv"]            # [D, 1]
    ctx_out = outs["ctx"]       # [B_L, F] f32
    attn_out = outs["attnw"]    # [B_L, T] f32

    # internal DRAM scratch for the (proj_hid + b1 + b2) rows
    ph_dram = nc.dram_tensor("ph_scratch", [B_L, D], F32).ap()

    # ---------------- pools ----------------
    const_pool = ctx.enter_context(tc.tile_pool(name="const", bufs=1))
    wpool = ctx.enter_context(tc.tile_pool(name="weights", bufs=1))
    natbf_pool = ctx.enter_context(tc.tile_pool(name="natbf", bufs=3))
    featT_pool = ctx.enter_context(tc.tile_pool(name="featT", bufs=2))
    tanh_pool = ctx.enter_context(tc.tile_pool(name="tanh", bufs=3))
    ttrs_pool = ctx.enter_context(tc.tile_pool(name="ttrs", bufs=2))
    sm_pool = ctx.enter_context(tc.tile_pool(name="smallsb", bufs=2))
    batch_pool = ctx.enter_context(tc.tile_pool(name="batchsb", bufs=2))

    ps_tr = ctx.enter_context(tc.tile_pool(name="ps_tr", bufs=2, space="PSUM"))
    ps_mm = ctx.enter_context(tc.tile_pool(name="ps_mm", bufs=2, space="PSUM"))
    ps_ctx = ctx.enter_context(tc.tile_pool(name="ps_ctx", bufs=1, space="PSUM"))

    # ---------------- constants ----------------
    ident_cdt = const_pool.tile([P, P], cdt, name="ident_cdt")
    masks.make_identity(nc, ident_cdt[:])
    ident_f32 = const_pool.tile([P, P], F32, name="ident_f32")
    masks.make_identity(nc, ident_f32[:])
    ones_f32 = const_pool.tile([P, P], F32, name="ones_f32")
    nc.gpsimd.memset(ones_f32[:], 1.0)
    ones_row = const_pool.tile([1, P], cdt, name="ones_row")
    nc.gpsimd.memset(ones_row[:], 1.0)

    def load_weight_cdt(dram_ap, kt, name):
        """Load [P, cols] slices of a [K, cols] weight, casting in the DMA."""
        tiles = []
        for k in range(kt):
            src = dram_ap[k * P:(k + 1) * P, :]
            cols = src.shape[1]
            wt = wpool.tile([P, cols], cdt, name=f"{name}_{k}")
            nc.gpsimd.dma_start(wt[:], src)
            tiles.append(wt)
        return tiles

    w1c = load_weight_cdt(w1_d, KF, "w1")
    w2c = load_weight_cdt(w2_d, KH, "w2")

    # Wv broadcast across partitions: [P, D] in cdt (cast in the DMA)
    wvbc = const_pool.tile([P, D], cdt, name="wvbc")
    nc.gpsimd.dma_start(wvbc[:], wv_d.rearrange("d o -> o d").to_broadcast((P, D)))

    # ---------------- stage 0: proj_hid = hidden @ W2 + b2 (+ b1) ----------------
    hid_sb = sm_pool.tile([B_L, H], F32, name="hid_sb")
    nc.sync.dma_start(hid_sb[:], hid_in_d[:, :])

    # transpose hidden -> hidT [P, B_L] per k-tile (cdt)
    hidT = []
    for kh in range(KH):
        pst = ps_tr.tile([P, B_L], F32, name="ps_tr")
        nc.tensor.matmul(
            pst[:], hid_sb[:, kh * P:(kh + 1) * P], ident_f32[:B_L, :B_L],
            is_transpose=True,
        )
        ht = sm_pool.tile([P, B_L], cdt, name=f"hidT_{kh}")
        nc.scalar.copy(ht[:], pst[:])
        hidT.append(ht)

    # proj_hid matmuls -> psum [B_L, 512] x ND2
    hidb = sm_pool.tile([B_L, D], F32, name="hidb")
    for nh in range(ND2):
        psph = ps_mm.tile([B_L, ND512], F32, name="ps_mm1")
        for kh in range(KH):
            nc.tensor.matmul(
                psph[:], hidT[kh][:], w2c[kh][:, nh * ND512:(nh + 1) * ND512],
                start=(kh == 0), stop=(kh == KH - 1),
            )
        nc.scalar.copy(hidb[:, nh * ND512:(nh + 1) * ND512], psph[:])

    # add b1 + b2 rows (broadcast-DMA'd to [B_L, D])
    for bias_d in (b1_d, b2_d):
        bb = sm_pool.tile([B_L, D], F32, name="biasb")
        nc.sync.dma_start(bb[:], bias_d[None, :].to_broadcast((B_L, D)))
        nc.vector.tensor_add(hidb[:], hidb[:], bb[:])

    nc.sync.dma_start(ph_dram[:, :], hidb[:])

    # ---------------- per-batch steady state ----------------
    for b in range(B_L):
        # bias row for this batch in cdt (cast in the DMA)
        phc = sm_pool.tile([1, D], cdt, name="phc")
        nc.gpsimd.dma_start(phc[:], ph_dram[b:b + 1, :])

        score_sb = batch_pool.tile([P, TS], F32, name="score")
        exps_sb = batch_pool.tile([P, TS], F32, name="exps")
        exps_cdt = batch_pool.tile([P, TS], cdt, name="exps_cdt")

        ctx_ps = [ps_ctx.tile([1, NF512], F32, name=f"ps_ctx{nf}") for nf in range(NF2)]

        for c in range(NC_):
            # ---- load natural chunk, casting f32->cdt in the DMA ----
            natbf = natbf_pool.tile([P, ST, F], cdt, name="natbf")
            nc.gpsimd.dma_start(
                natbf[:],
                feat_in[b, c * CHUNK:(c + 1) * CHUNK, :].rearrange(
                    "(st p) f -> p st f", p=P
                ),
            )

            # ---- transpose to featT [P(f), KF, CHUNK(t)], 2 kf per psum bank ----
            featT = featT_pool.tile([P, KF, CHUNK], cdt, name="featT")
            for kfp in range(KF // 2):
                pst = ps_tr.tile([P, 2 * CHUNK], cdt, name="ps_tr")
                for kfl in range(2):
                    kf = 2 * kfp + kfl
                    for st in range(ST):
                        nc.tensor.matmul(
                            pst[:, kfl * CHUNK + st * P: kfl * CHUNK + (st + 1) * P],
                            natbf[:, st, kf * P:(kf + 1) * P],
                            ident_cdt[:],
                            is_transpose=True,
                        )
                nc.scalar.copy(
                    featT[:, 2 * kfp:2 * kfp + 2, :].rearrange("p a b -> p (a b)"),
                    pst[:],
                )

            # ---- mm1 + tanh + score ----
            for st in range(ST):
                tanhc = tanh_pool.tile([P, D], cdt, name="tanhc")
                pso = ps_mm.tile([P, D], F32, name="ps_mm1")
                for nd in range(ND2):
                    # K=1 bias matmul: ones[128] (x) ph_row -> broadcast add
                    nc.tensor.matmul(
                        pso[:, nd * ND512:(nd + 1) * ND512],
                        ones_row[:], phc[:, nd * ND512:(nd + 1) * ND512],
                        start=True, stop=False,
                    )
                    for kf in range(KF):
                        nc.tensor.matmul(
                            pso[:, nd * ND512:(nd + 1) * ND512],
                            featT[:, kf, st * P:(st + 1) * P],
                            w1c[kf][:, nd * ND512:(nd + 1) * ND512],
                            start=False, stop=(kf == KF - 1),
                        )
                nc.scalar.activation(
                    tanhc[:], pso[:], mybir.ActivationFunctionType.Tanh,
                )
                # score col = sum_d tanh * Wv
                col = c * ST + st
                ttrs = ttrs_pool.tile([P, D], cdt, name="ttrs")
                nc.vector.scalar_tensor_tensor(
                    out=ttrs[:], in0=tanhc[:], scalar=0.0, in1=wvbc[:],
                    op0=mybir.AluOpType.bypass, op1=mybir.AluOpType.mult,
                    accum_out=score_sb[:, col:col + 1],
                )

            # ---- exp of this chunk's score cols ----
            nc.scalar.activation(
                exps_sb[:, c * ST:(c + 1) * ST], score_sb[:, c * ST:(c + 1) * ST],
                mybir.ActivationFunctionType.Exp,
            )
            nc.vector.tensor_copy(
                exps_cdt[:, c * ST:(c + 1) * ST], exps_sb[:, c * ST:(c + 1) * ST]
            )

            # ---- mm2 partial: ctx_psum += exp_col.T @ nat_chunk ----
            for st in range(ST):
                col = c * ST + st
                for nf in range(NF2):
                    nc.tensor.matmul(
                        ctx_ps[nf][:],
                        exps_cdt[:, col:col + 1],
                        natbf[:, st, nf * NF512:(nf + 1) * NF512],
                        start=(c == 0 and st == 0),
                        stop=(c == NC_ - 1 and st == ST - 1),
                    )

        # ---------------- batch epilogue: softmax normalize ----------------
        rowsum = sm_pool.tile([P, 1], F32, name="rowsum")
        nc.vector.reduce_sum(rowsum[:], exps_sb[:], axis=mybir.AxisListType.X)
        ps_tot = ps_tr.tile([P, 1], F32, name="ps_tr")
        nc.tensor.matmul(ps_tot[:], ones_f32[:], rowsum[:])
        rtotb = sm_pool.tile([P, 1], F32, name="rtotb")
        nc.vector.reciprocal(rtotb[:], ps_tot[:])

        # normalized attention weights, transposed for contiguous DMA out
        attnw = batch_pool.tile([P, TS], F32, name="attnw")
        nc.scalar.mul(attnw[:], exps_sb[:], rtotb[:])
        ps_at = ps_tr.tile([TS, P], F32, name="ps_tr")
        nc.tensor.matmul(ps_at[:], attnw[:], ident_f32[:], is_transpose=True)
        at_sb = sm_pool.tile([TS, P], F32, name="at_sb")
        nc.scalar.copy(at_sb[:], ps_at[:])
        nc.sync.dma_start(
            attn_out[b].rearrange("(j p) -> j p", p=P), at_sb[:]
        )

        # ctx = ctx_psum / total
        ctx_sb = sm_pool.tile([1, F], F32, name="ctx_sb")
        for nf in range(NF2):
            nc.scalar.mul(
                ctx_sb[:, nf * NF512:(nf + 1) * NF512], ctx_ps[nf][:],
                rtotb[0:1, 0:1],
            )
        nc.sync.dma_start(ctx_out[b:b + 1, :], ctx_sb[:])

    ctx.close()


def build_nc(
    B_L=B_L, T=T, F=F, H=H, D=DIM, CHUNK=512, cdt=mybir.dt.bfloat16,
    tile_kwargs=None,
):
    """Build + schedule + compile the per-core Bass module."""
    nc = bacc.Bacc("TRN2", target_bir_lowering=False, debug=False,
                   num_devices=N_CORES)
    ins = {
        "hidden": nc.dram_tensor("hidden", [B_L, H], F32, kind="ExternalInput").ap(),
        "features": nc.dram_tensor("features", [B_L, T, F], F32, kind="ExternalInput").ap(),
        "W1": nc.dram_tensor("W1", [F, D], F32, kind="ExternalInput").ap(),
        "b1": nc.dram_tensor("b1", [D], F32, kind="ExternalInput").ap(),
        "W2": nc.dram_tensor("W2", [H, D], F32, kind="ExternalInput").ap(),
        "b2": nc.dram_tensor("b2", [D], F32, kind="ExternalInput").ap(),
        "Wv": nc.dram_tensor("Wv", [D, 1], F32, kind="ExternalInput").ap(),
        "bv": nc.dram_tensor("bv", [1], F32, kind="ExternalInput").ap(),
    }
    outs = {
        "ctx": nc.dram_tensor("ctx", [B_L, F], F32, kind="ExternalOutput").ap(),
        "attnw": nc.dram_tensor("attnw", [B_L, T], F32, kind="ExternalOutput").ap(),
    }
    with tile.TileContext(nc, **(tile_kwargs or {})) as tc:
        build_attention_kernel(nc, tc, ins, outs, B_L, T, F, H, D, CHUNK, cdt)
    nc.compile()
    return nc


_nc_cache = None

# test-harness knobs (the grading harness just calls kernel(); these stay default)
TRACE = False
LAST_RESULTS = None


def kernel(hidden, features, W1, b1, W2, b2, Wv, bv):
    global _nc_cache, LAST_RESULTS
    if _nc_cache is None:
        _nc_cache = build_nc()
    nc = _nc_cache

    hidden = np.ascontiguousarray(hidden, dtype=np.float32)
    features = np.ascontiguousarray(features, dtype=np.float32)
    in_common = {
        "W1": np.ascontiguousarray(W1, np.float32),
        "b1": np.ascontiguousarray(b1, np.float32),
        "W2": np.ascontiguousarray(W2, np.float32),
        "b2": np.ascontiguousarray(b2, np.float32),
        "Wv": np.ascontiguousarray(Wv, np.float32),
        "bv": np.ascontiguousarray(bv, np.float32),
    }
    in_maps = []
    for i in range(N_CORES):
        sl = slice(i * B_L, (i + 1) * B_L)
        in_maps.append({
            "hidden": hidden[sl], "features": features[sl], **in_common,
        })

    res = run_bass_kernel_spmd(nc, in_maps, core_ids=list(range(N_CORES)),
                               trace=TRACE)
    LAST_RESULTS = res
    ctx = np.concatenate([r["ctx"] for r in res.results], axis=0)
    attn = np.concatenate([r["attnw"] for r in res.results], axis=0)[..., None]
    return ctx.astype(np.float32), attn.astype(np.float32)
